# revision 5
# baseline (speedup 1.0000x reference)
"""Trainium2 Bass kernel for the Bengio03 Highway BiLM problem.

Math (see reference): L=3 layers, each with fwd/bwd chains. Per (layer, dir):
  padded = [front_pads(4), seq(512), back_pads(4)]          # [B, 520, H]
  pre[t] = sum_{k=0..4} padded[t + k + off] @ W[k*H:(k+1)*H]  (off=0 fwd, 4 bwd)
  x0 = relu(pre + b)
  2x highway: proj = x @ Ws[j] + bs[j]; nonlin,gate = split(proj)
              x = sigmoid(gate)*x + (1-sigmoid(gate))*relu(nonlin)
  out[l,:,:, 0:512] = f chain, [512:1024] = b chain

Implementation strategy (per core, data-parallel over batch: 4 seqs/core):
  - All matmul operands in fp16 (fp32 PSUM accumulation): same PE rate as
    bf16 (1 cycle/row) but 8x less rounding noise (10-bit vs 7-bit mantissa).
  - Activations kept feature-major in SBUF, one tile per sequence holding all
    4 hidden chunks: [128(h), 4(chunk), 520(t)] (pads inline). Matmuls compute
    outT = W_tile.T @ xT directly (weights stationary lhsT, activations stream
    as rhs with N=512 tokens); the 5-tap conv is 5 shifted rhs slices
    accumulated in PSUM (20 matmuls of [128,128]@[128,512] per output chunk).
  - Layer-0 input arrives host-padded and pre-transposed; output is stored
    feature-major fp16 and re-transposed on the host.
  - DMAs are merged into few large transfers (host pre-packs weights in SBUF
    layout): 4 n-waves per (l,dir) conv tensor, 1 per (l,dir,j) highway
    tensor, 1 per input sequence, 1 per output tile (~75 triggers total,
    ~650ns each on the issuing engine), round-robined over sync/gpsimd so
    scalar/vector queues stay clean.
  - A short warm-up matmul train runs while the first DMAs land so the PE
    clock is at 2.4GHz (not the 1.2GHz cold pstate) when real work starts.
  - Final-layer bwd stores are emitted per chunk so the drain overlaps the
    last compute instead of serializing after it.
  - Elementwise: VectorE does conv relu + the 3 highway-combine ops, ScalarE
    does relu/sigmoid of the highway (+bias, PSUM->SBUF), GpSimd writes pad
    columns for layer>0 activations.
"""

import os
import sys

sys.path.insert(0, "/opt/trn_rl_repo")

import numpy as np

import concourse.bass as bass
import concourse.bacc as bacc
import concourse.tile as tile
from concourse import mybir
from concourse.bass_utils import run_bass_kernel_spmd

# Problem constants (hardcoded per spec).
L = 3
WIDTH = 4
H = 512
B = 32
S = 512
NHW = 2
CIN = (WIDTH + 1) * H  # 2560
NCORES = 8
BLOC = B // NCORES  # 4 sequences per core
HC = H // 128  # 4 hidden chunks of 128
SPAD = S + 2 * WIDTH  # 520
NR = CIN // 128  # 20 contraction chunks for the conv
F32 = mybir.dt.float32
FP16 = mybir.dt.float16
RELU = mybir.ActivationFunctionType.Relu
SIGM = mybir.ActivationFunctionType.Sigmoid
ADD = mybir.AluOpType.add
MAX = mybir.AluOpType.max


def _build_program():
    nc = bacc.Bacc(
        "TRN2",
        target_bir_lowering=False,
        debug=False,
        enable_asserts=False,
        num_devices=1,
    )

    # Layer-0 input, host-padded + feature-major: [b, 128, chunk, 520]
    xT_d = nc.dram_tensor("xT", [BLOC, 128, HC, SPAD], FP16, kind="ExternalInput").ap()
    # Conv weights in SBUF layout: [l, 128, n(out chunk), r(contraction), 128]
    fw_d = nc.dram_tensor("fw", [L, 128, HC, NR, 128], FP16, kind="ExternalInput").ap()
    bw_d = nc.dram_tensor("bw", [L, 128, HC, NR, 128], FP16, kind="ExternalInput").ap()
    # Highway weights: [l, j, 128, h(4), 2H]
    fhw_d = nc.dram_tensor("fhw", [L, NHW, 128, HC, 2 * H], FP16, kind="ExternalInput").ap()
    bhw_d = nc.dram_tensor("bhw", [L, NHW, 128, HC, 2 * H], FP16, kind="ExternalInput").ap()
    # Biases host-packed into single planes: [128, L*HC] / [128, L*NHW*2HC]
    cb_d = nc.dram_tensor("cb", [2, 128, L * HC], F32, kind="ExternalInput").ap()
    hb_d = nc.dram_tensor("hb", [2, 128, L * NHW * 2 * HC], F32, kind="ExternalInput").ap()
    # Pads host-packed: [2, 128, l, chunk, w]
    pad_d = nc.dram_tensor("pad", [2, 128, L, HC, WIDTH], FP16, kind="ExternalInput").ap()
    # Output feature-major fp16: [l, b, dir, 128, chunk, t]; host transposes.
    out_d = nc.dram_tensor("out", [L, BLOC, 2, 128, HC, S], FP16, kind="ExternalOutput").ap()

    with tile.TileContext(nc) as tc:
        with (
            tc.tile_pool(name="consts", bufs=1) as consts,
            tc.tile_pool(name="acts", bufs=12) as acts,
            tc.tile_pool(name="outs", bufs=5) as outsp,
            tc.tile_pool(name="convw", bufs=3) as convw,
            tc.tile_pool(name="hww", bufs=3) as hww,
            tc.tile_pool(name="xmid", bufs=8) as xmid,
            tc.tile_pool(name="work", bufs=4) as work,
            tc.tile_pool(name="psum", bufs=2, space="PSUM") as psum,
        ):
            QS = (nc.sync, nc.gpsimd)
            qctr = [0]

            def qnext():
                q = QS[qctr[0] % len(QS)]
                qctr[0] += 1
                return q

            # ---- PE warm-up: dummy matmuls on a memset tile while the ----
            # ---- critical input/weight DMAs land (cold pstate is 1.2GHz).
            # The train is sized to bridge until the first input+weight
            # DMAs complete (~20us incl. runtime init) so the PE hits the
            # first real matmul already at full clock with no idle gap.
            warm = consts.tile([128, 512], FP16, name="warm", tag="warm", bufs=1)
            nc.gpsimd.memset(warm[:], 0.0)
            wps = psum.tile([128, S], F32, name="wps", tag="cpsum", bufs=4)
            for i in range(34):
                nc.tensor.matmul(
                    wps[:], lhsT=warm[:, 0:128], rhs=warm[:],
                    start=True, stop=True,
                )

            # ---- tiles for constants (DMAs issued in the startup block) ----
            padt = consts.tile([128, 2, L, HC, WIDTH], FP16, name="padt", tag="padt", bufs=1)
            cbt = consts.tile([128, 2, L * HC], F32, name="cbt", tag="cbt", bufs=1)
            hbt = consts.tile([128, 2, L * NHW * 2 * HC], F32, name="hbt", tag="hbt", bufs=1)

            def cbias(dirc, l, n):
                return cbt[:, 0 if dirc == "f" else 1, l * HC + n: l * HC + n + 1]

            def hbias(dirc, l, j, c):
                o = (l * NHW + j) * 2 * HC + c
                return hbt[:, 0 if dirc == "f" else 1, o:o + 1]

            def write_pads(at, l):
                # at: [128, HC, SPAD]; fill cols [0:4] and [516:520] per chunk
                nc.gpsimd.tensor_copy(at[:, :, 0:WIDTH], padt[:, 0, l])
                nc.gpsimd.tensor_copy(at[:, :, WIDTH + S:SPAD], padt[:, 1, l])

            loaded_cw = {}
            loaded_hw = {}

            def load_convw(dirc, l, split=False):
                # One [128, HC, NR, 128] tile per (dir, l); DMA'd in 4 n-waves
                # (each wave is everything output-chunk n needs).
                src = fw_d if dirc == "f" else bw_d
                w = convw.tile([128, HC, NR, 128], FP16,
                               name=f"cw_{dirc}{l}", tag="convw", bufs=3)
                if not split:
                    for n in range(HC):
                        qnext().dma_start(w[:, n], src[l][:, n])
                loaded_cw[(dirc, l)] = w
                return w

            def ensure_convw(dirc, l):
                if (dirc, l) not in loaded_cw:
                    load_convw(dirc, l)
                return loaded_cw[(dirc, l)]

            def ensure_hww(dirc, l):
                if (dirc, l) not in loaded_hw:
                    src = fhw_d if dirc == "f" else bhw_d
                    jt = []
                    for j in range(NHW):
                        w = hww.tile([128, HC, 2 * H], FP16,
                                     name=f"hw_{dirc}{l}_{j}", tag="hww", bufs=3)
                        qnext().dma_start(w[:], src[l, j])
                        jt.append(w)
                    loaded_hw[(dirc, l)] = jt
                return loaded_hw[(dirc, l)]

            # ---- startup: interleave the critical loads across both queues
            # so group (b0,n0) [needs x0+w0] can start after ~1.2MB lands.
            xT = {}
            for b in range(BLOC):
                xT[b] = acts.tile([128, HC, SPAD], FP16, name=f"xT_{b}", tag="acts", bufs=12)
            cwf0 = load_convw("f", 0, split=True)
            nc.sync.dma_start(xT[0][:], xT_d[0])
            nc.gpsimd.dma_start(xT[1][:], xT_d[1])
            nc.sync.dma_start(cwf0[:, 0], fw_d[0][:, 0])
            nc.gpsimd.dma_start(cwf0[:, 1], fw_d[0][:, 1])
            nc.sync.dma_start(xT[2][:], xT_d[2])
            nc.gpsimd.dma_start(xT[3][:], xT_d[3])
            nc.sync.dma_start(cwf0[:, 2], fw_d[0][:, 2])
            nc.gpsimd.dma_start(cwf0[:, 3], fw_d[0][:, 3])
            nc.sync.dma_start(cbt[:, 0], cb_d[0])
            nc.gpsimd.dma_start(cbt[:, 1], cb_d[1])
            nc.sync.dma_start(padt[:, 0], pad_d[0])
            nc.gpsimd.dma_start(padt[:, 1], pad_d[1])
            nc.sync.dma_start(hbt[:, 0], hb_d[0])
            nc.gpsimd.dma_start(hbt[:, 1], hb_d[1])

            # ---- stages ----
            def conv_stage(dirc, l, srcset, pair, wt, n_outer=False):
                off0 = 0 if dirc == "f" else WIDTH
                x0 = {}
                if n_outer:  # startup: early groups only need early n-waves
                    groups = [(b, n) for n in range(HC) for b in pair]
                else:
                    groups = [(b, n) for b in pair for n in range(HC)]
                for b, n in groups:
                    if b not in x0:
                        x0[b] = xmid.tile([128, HC, S], FP16, name=f"x0_{b}", tag="xmid", bufs=8)
                    ps = psum.tile([128, S], F32, name=f"cps_{b}_{n}", tag="cpsum", bufs=4)
                    for r in range(NR):
                        k, ci = divmod(r, HC)
                        off = off0 + k
                        nc.tensor.matmul(
                            ps[:],
                            lhsT=wt[:, n, r],
                            rhs=srcset[b][:, ci, off:off + S],
                            start=(r == 0),
                            stop=(r == NR - 1),
                        )
                    nc.vector.tensor_scalar(
                        x0[b][:, n], ps[:], cbias(dirc, l, n), 0.0, ADD, MAX
                    )
                return x0

            def hw_stage(dirc, l, j, srcset, pair, wt, final):
                outs = {}
                last = dirc == "b" and l == L - 1
                for b in pair:
                    if final:
                        if l + 1 < L:
                            at = acts.tile([128, HC, SPAD], FP16,
                                           name=f"a_{dirc}{l}_{b}", tag="acts", bufs=12)
                            write_pads(at, l + 1)
                        else:
                            at = outsp.tile([128, HC, S], FP16,
                                            name=f"o_{dirc}{l}_{b}", tag="outs", bufs=5)
                        outs[b] = at
                    else:
                        outs[b] = xmid.tile([128, HC, S], FP16, name=f"x1_{b}", tag="xmid", bufs=8)
                    for c in range(HC):
                        pnl = psum.tile([128, S], F32, name=f"hnl_{b}_{c}", tag="hpsum", bufs=4)
                        for h in range(HC):
                            nc.tensor.matmul(
                                pnl[:],
                                lhsT=wt[:, h, c * 128:(c + 1) * 128],
                                rhs=srcset[b][:, h],
                                start=(h == 0),
                                stop=(h == HC - 1),
                            )
                        pgt = psum.tile([128, S], F32, name=f"hgt_{b}_{c}", tag="hpsum", bufs=4)
                        for h in range(HC):
                            nc.tensor.matmul(
                                pgt[:],
                                lhsT=wt[:, h, H + c * 128:H + (c + 1) * 128],
                                rhs=srcset[b][:, h],
                                start=(h == 0),
                                stop=(h == HC - 1),
                            )
                        r = work.tile([128, S], FP16, name=f"r_{b}_{c}", tag="r", bufs=4)
                        nc.scalar.activation(r[:], pnl[:], RELU, bias=hbias(dirc, l, j, c))
                        g = work.tile([128, S], FP16, name=f"g_{b}_{c}", tag="g", bufs=4)
                        nc.scalar.activation(g[:], pgt[:], SIGM, bias=hbias(dirc, l, j, HC + c))
                        d = work.tile([128, S], FP16, name=f"d_{b}_{c}", tag="d", bufs=4)
                        nc.vector.tensor_sub(d[:], srcset[b][:, c], r[:])
                        nc.vector.tensor_mul(d[:], g[:], d[:])
                        if final:
                            if l + 1 < L:
                                dst = outs[b][:, c, WIDTH:WIDTH + S]
                            else:
                                dst = outs[b][:, c]
                            nc.vector.tensor_add(dst, d[:], r[:])
                            if last:
                                # fine-grained final drain: store each chunk
                                # as soon as its combine lands
                                QS[(c + b) % len(QS)].dma_start(out_d[l, b, 1][:, c], dst)
                        else:
                            nc.vector.tensor_add(outs[b][:, c], d[:], r[:])
                if final and not last:
                    k = 0 if dirc == "f" else 1
                    for b in pair:
                        src = outs[b][:, :, WIDTH:WIDTH + S] if l + 1 < L else outs[b][:]
                        qnext().dma_start(out_d[l, b, k], src)
                return outs

            # ---- main chain: f fully, then b (xT stays resident for b) ----
            PAIRS = [(0, 1), (2, 3)]
            first = True
            for dirc in ("f", "b"):
                cur = xT
                for l in range(L):
                    cw = ensure_convw(dirc, l)
                    nxt = {}
                    hw = None
                    for pair in PAIRS:
                        x0 = conv_stage(dirc, l, cur, pair, cw, n_outer=first)
                        # hww DMAs emitted after the first conv groups so they
                        # don't compete with the critical startup loads; also
                        # prefetch the next conv weights here.
                        if hw is None:
                            hw = ensure_hww(dirc, l)
                            nl = (dirc, l + 1) if l + 1 < L else ("b", 0)
                            if nl not in loaded_cw:
                                ensure_convw(*nl)
                        x1 = hw_stage(dirc, l, 0, x0, pair, hw[0], final=False)
                        res = hw_stage(dirc, l, 1, x1, pair, hw[1], final=True)
                        nxt.update(res)
                    first = False
                    cur = nxt

    nc.compile()
    return nc


_CACHE = {}


def _get_program():
    if "nc" not in _CACHE:
        _CACHE["nc"] = _build_program()
    return _CACHE["nc"]


def _make_in_maps(inputs):
    f16 = np.float16

    # Conv weights [L, CIN, H] -> [L, 128, n, r, 128]
    def packw(w):
        w = np.asarray(w, dtype=np.float32).astype(f16)
        # CIN index = r*128 + kp ; H index = n*128 + m
        w = w.reshape(L, NR, 128, HC, 128)           # [l, r, kp, n, m]
        return np.ascontiguousarray(w.transpose(0, 2, 3, 1, 4))  # [l, kp, n, r, m]

    # Highway weights [L, NHW, H, 2H] -> [L, NHW, 128, h, 2H]
    def packhw(w):
        w = np.asarray(w, dtype=np.float32).astype(f16)
        w = w.reshape(L, NHW, HC, 128, 2 * H)        # [l, j, h, kp, 2H]
        return np.ascontiguousarray(w.transpose(0, 1, 3, 2, 4))  # [l, j, kp, h, 2H]

    fw = packw(inputs["fwd_W"])
    bw = packw(inputs["bwd_W"])
    fhw = packhw(inputs["fwd_hw_W"])
    bhw = packhw(inputs["bwd_hw_W"])

    # Conv biases [L, H] -> [128, L*HC] stacked f/b
    def packcb(b):
        b = np.asarray(b, dtype=np.float32).reshape(L, HC, 128).transpose(2, 0, 1)
        return b.reshape(128, L * HC)

    cb = np.ascontiguousarray(np.stack([packcb(inputs["fwd_b"]), packcb(inputs["bwd_b"])]))

    # Highway biases [L, NHW, 2H] -> [128, L*NHW*2HC] stacked f/b
    def packhb(b):
        b = np.asarray(b, dtype=np.float32).reshape(L, NHW, 2 * HC, 128).transpose(3, 0, 1, 2)
        return b.reshape(128, L * NHW * 2 * HC)

    hb = np.ascontiguousarray(np.stack([packhb(inputs["fwd_hw_b"]), packhb(inputs["bwd_hw_b"])]))

    # Pads [L, W, H] -> [128, L, HC, W] stacked f/b
    def packpad(p):
        p = np.asarray(p, dtype=np.float32).reshape(L, WIDTH, HC, 128).transpose(3, 0, 2, 1)
        return np.ascontiguousarray(p.astype(f16))   # [128, L, HC, W]

    pad = np.ascontiguousarray(np.stack([packpad(inputs["fwd_pads"]), packpad(inputs["bwd_pads"])]))

    # Layer-0 input: [B, S, H] -> per core [BLOC, 128, HC, SPAD] host-padded
    x = np.asarray(inputs["inputs"], dtype=np.float32).astype(f16)
    fpads0 = np.asarray(inputs["fwd_pads"], dtype=np.float32)[0].astype(f16)  # [W, H]
    bpads0 = np.asarray(inputs["bwd_pads"], dtype=np.float32)[0].astype(f16)

    shared = {
        "fw": fw, "bw": bw, "fhw": fhw, "bhw": bhw,
        "cb": cb, "hb": hb, "pad": pad,
    }
    in_maps = []
    for i in range(NCORES):
        m = dict(shared)
        xi = x[i * BLOC:(i + 1) * BLOC]              # [BLOC, S, H]
        xp = np.empty((BLOC, SPAD, H), dtype=f16)
        xp[:, WIDTH:WIDTH + S] = xi
        xp[:, :WIDTH] = fpads0
        xp[:, WIDTH + S:] = bpads0
        # [BLOC, SPAD, (HC,128)] -> [BLOC, 128, HC, SPAD]
        xp = xp.reshape(BLOC, SPAD, HC, 128).transpose(0, 3, 2, 1)
        m["xT"] = np.ascontiguousarray(xp)
        in_maps.append(m)
    return in_maps


def _run(inputs, trace=False, tmpdir=None):
    nc = _get_program()
    in_maps = _make_in_maps(inputs)
    res = run_bass_kernel_spmd(
        nc, in_maps, core_ids=list(range(NCORES)), trace=trace, tmpdir=tmpdir
    )
    # [L, BLOC, 2, 128, HC, S] per core -> concat on batch -> [L, B, S, 2H] fp32
    out = np.concatenate([np.asarray(r["out"]) for r in res.results], axis=1)
    # [l, b, dir, p, c, t] -> [l, b, t, dir, c, p]
    out = out.transpose(0, 1, 5, 2, 4, 3).reshape(L, B, S, 2 * H).astype(np.float32)
    return out, res


def kernel(**inputs):
    trace = bool(int(os.environ.get("BASS_KERNEL_TRACE", "0")))
    out, _ = _run(inputs, trace=trace)
    return out


# revision 6
# speedup vs baseline: 1.0012x; 1.0012x over previous
"""Trainium2 Bass kernel for the Bengio03 Highway BiLM problem.

Math (see reference): L=3 layers, each with fwd/bwd chains. Per (layer, dir):
  padded = [front_pads(4), seq(512), back_pads(4)]          # [B, 520, H]
  pre[t] = sum_{k=0..4} padded[t + k + off] @ W[k*H:(k+1)*H]  (off=0 fwd, 4 bwd)
  x0 = relu(pre + b)
  2x highway: proj = x @ Ws[j] + bs[j]; nonlin,gate = split(proj)
              x = sigmoid(gate)*x + (1-sigmoid(gate))*relu(nonlin)
  out[l,:,:, 0:512] = f chain, [512:1024] = b chain

Implementation strategy (per core, data-parallel over batch: 4 seqs/core):
  - All matmul operands in fp16 (fp32 PSUM accumulation): same PE rate as
    bf16 (1 cycle/row) but 8x less rounding noise (10-bit vs 7-bit mantissa).
  - Activations kept feature-major in SBUF, one tile per sequence holding all
    4 hidden chunks: [128(h), 4(chunk), 520(t)] (pads inline). Matmuls compute
    outT = W_tile.T @ xT directly (weights stationary lhsT, activations stream
    as rhs with N=512 tokens); the 5-tap conv is 5 shifted rhs slices
    accumulated in PSUM (20 matmuls of [128,128]@[128,512] per output chunk).
  - Layer-0 input arrives host-padded and pre-transposed; output is stored
    feature-major fp16 and re-transposed on the host.
  - DMAs are merged into few large transfers (host pre-packs weights in SBUF
    layout): 4 n-waves per (l,dir) conv tensor, 1 per (l,dir,j) highway
    tensor, 1 per input sequence, 1 per output tile (~75 triggers total,
    ~650ns each on the issuing engine), round-robined over sync/gpsimd so
    scalar/vector queues stay clean.
  - A short warm-up matmul train runs while the first DMAs land so the PE
    clock is at 2.4GHz (not the 1.2GHz cold pstate) when real work starts.
  - Final-layer bwd stores are emitted per chunk so the drain overlaps the
    last compute instead of serializing after it.
  - Elementwise: VectorE does conv relu + the 3 highway-combine ops, ScalarE
    does relu/sigmoid of the highway (+bias, PSUM->SBUF), GpSimd writes pad
    columns for layer>0 activations.
"""

import os
import sys

sys.path.insert(0, "/opt/trn_rl_repo")

import numpy as np

import concourse.bass as bass
import concourse.bacc as bacc
import concourse.tile as tile
from concourse import mybir
from concourse.bass_utils import run_bass_kernel_spmd

# Problem constants (hardcoded per spec).
L = 3
WIDTH = 4
H = 512
B = 32
S = 512
NHW = 2
CIN = (WIDTH + 1) * H  # 2560
NCORES = 8
BLOC = B // NCORES  # 4 sequences per core
HC = H // 128  # 4 hidden chunks of 128
SPAD = S + 2 * WIDTH  # 520
NR = CIN // 128  # 20 contraction chunks for the conv
F32 = mybir.dt.float32
FP16 = mybir.dt.float16
RELU = mybir.ActivationFunctionType.Relu
SIGM = mybir.ActivationFunctionType.Sigmoid
ADD = mybir.AluOpType.add
MAX = mybir.AluOpType.max


def _build_program():
    nc = bacc.Bacc(
        "TRN2",
        target_bir_lowering=False,
        debug=False,
        enable_asserts=False,
        num_devices=1,
    )

    # Layer-0 input, host-padded + feature-major: [b, 128, chunk, 520]
    xT_d = nc.dram_tensor("xT", [BLOC, 128, HC, SPAD], FP16, kind="ExternalInput").ap()
    # Conv weights in SBUF layout: [l, 128, n(out chunk), r(contraction), 128]
    fw_d = nc.dram_tensor("fw", [L, 128, HC, NR, 128], FP16, kind="ExternalInput").ap()
    bw_d = nc.dram_tensor("bw", [L, 128, HC, NR, 128], FP16, kind="ExternalInput").ap()
    # Highway weights: [l, j, 128, h(4), 2H]
    fhw_d = nc.dram_tensor("fhw", [L, NHW, 128, HC, 2 * H], FP16, kind="ExternalInput").ap()
    bhw_d = nc.dram_tensor("bhw", [L, NHW, 128, HC, 2 * H], FP16, kind="ExternalInput").ap()
    # Biases host-packed into single planes: [128, L*HC] / [128, L*NHW*2HC]
    cb_d = nc.dram_tensor("cb", [2, 128, L * HC], F32, kind="ExternalInput").ap()
    hb_d = nc.dram_tensor("hb", [2, 128, L * NHW * 2 * HC], F32, kind="ExternalInput").ap()
    # Pads host-packed: [2, 128, l, chunk, w]
    pad_d = nc.dram_tensor("pad", [2, 128, L, HC, WIDTH], FP16, kind="ExternalInput").ap()
    # Output feature-major fp16: [l, b, dir, 128, chunk, t]; host transposes.
    out_d = nc.dram_tensor("out", [L, BLOC, 2, 128, HC, S], FP16, kind="ExternalOutput").ap()

    with tile.TileContext(nc) as tc:
        with (
            tc.tile_pool(name="consts", bufs=1) as consts,
            tc.tile_pool(name="acts", bufs=12) as acts,
            tc.tile_pool(name="outs", bufs=5) as outsp,
            tc.tile_pool(name="convw", bufs=3) as convw,
            tc.tile_pool(name="hww", bufs=3) as hww,
            tc.tile_pool(name="xmid", bufs=8) as xmid,
            tc.tile_pool(name="work", bufs=4) as work,
            tc.tile_pool(name="psum", bufs=2, space="PSUM") as psum,
        ):
            QS = (nc.sync, nc.gpsimd)
            qctr = [0]

            def qnext():
                q = QS[qctr[0] % len(QS)]
                qctr[0] += 1
                return q

            # ---- PE warm-up: dummy matmuls on a memset tile while the ----
            # ---- critical input/weight DMAs land (cold pstate is 1.2GHz).
            # The train is sized to bridge until the first input+weight
            # DMAs complete (~20us incl. runtime init) so the PE hits the
            # first real matmul already at full clock with no idle gap.
            warm = consts.tile([128, 512], FP16, name="warm", tag="warm", bufs=1)
            nc.gpsimd.memset(warm[:], 0.0)
            wps = psum.tile([128, S], F32, name="wps", tag="cpsum", bufs=4)
            for i in range(30):
                nc.tensor.matmul(
                    wps[:], lhsT=warm[:, 0:128], rhs=warm[:],
                    start=True, stop=True,
                )

            # ---- tiles for constants (DMAs issued in the startup block) ----
            padt = consts.tile([128, 2, L, HC, WIDTH], FP16, name="padt", tag="padt", bufs=1)
            cbt = consts.tile([128, 2, L * HC], F32, name="cbt", tag="cbt", bufs=1)
            hbt = consts.tile([128, 2, L * NHW * 2 * HC], F32, name="hbt", tag="hbt", bufs=1)

            def cbias(dirc, l, n):
                return cbt[:, 0 if dirc == "f" else 1, l * HC + n: l * HC + n + 1]

            def hbias(dirc, l, j, c):
                o = (l * NHW + j) * 2 * HC + c
                return hbt[:, 0 if dirc == "f" else 1, o:o + 1]

            def write_pads(at, l):
                # at: [128, HC, SPAD]; fill cols [0:4] and [516:520] per chunk
                nc.gpsimd.tensor_copy(at[:, :, 0:WIDTH], padt[:, 0, l])
                nc.gpsimd.tensor_copy(at[:, :, WIDTH + S:SPAD], padt[:, 1, l])

            loaded_cw = {}
            loaded_hw = {}

            def load_convw(dirc, l, split=False):
                # One [128, HC, NR, 128] tile per (dir, l); DMA'd in 4 n-waves
                # (each wave is everything output-chunk n needs).
                src = fw_d if dirc == "f" else bw_d
                w = convw.tile([128, HC, NR, 128], FP16,
                               name=f"cw_{dirc}{l}", tag="convw", bufs=3)
                if not split:
                    for n in range(HC):
                        qnext().dma_start(w[:, n], src[l][:, n])
                loaded_cw[(dirc, l)] = w
                return w

            def ensure_convw(dirc, l):
                if (dirc, l) not in loaded_cw:
                    load_convw(dirc, l)
                return loaded_cw[(dirc, l)]

            def ensure_hww(dirc, l):
                if (dirc, l) not in loaded_hw:
                    src = fhw_d if dirc == "f" else bhw_d
                    jt = []
                    for j in range(NHW):
                        w = hww.tile([128, HC, 2 * H], FP16,
                                     name=f"hw_{dirc}{l}_{j}", tag="hww", bufs=3)
                        qnext().dma_start(w[:], src[l, j])
                        jt.append(w)
                    loaded_hw[(dirc, l)] = jt
                return loaded_hw[(dirc, l)]

            # ---- startup: interleave the critical loads across both queues
            # so group (b0,n0) [needs x0+w0] can start after ~1.2MB lands.
            xT = {}
            for b in range(BLOC):
                xT[b] = acts.tile([128, HC, SPAD], FP16, name=f"xT_{b}", tag="acts", bufs=12)
            cwf0 = load_convw("f", 0, split=True)
            nc.sync.dma_start(xT[0][:], xT_d[0])
            nc.gpsimd.dma_start(xT[1][:], xT_d[1])
            nc.sync.dma_start(cwf0[:, 0], fw_d[0][:, 0])
            nc.gpsimd.dma_start(cwf0[:, 1], fw_d[0][:, 1])
            nc.sync.dma_start(xT[2][:], xT_d[2])
            nc.gpsimd.dma_start(xT[3][:], xT_d[3])
            nc.sync.dma_start(cwf0[:, 2], fw_d[0][:, 2])
            nc.gpsimd.dma_start(cwf0[:, 3], fw_d[0][:, 3])
            nc.sync.dma_start(cbt[:, 0], cb_d[0])
            nc.gpsimd.dma_start(cbt[:, 1], cb_d[1])
            nc.sync.dma_start(padt[:, 0], pad_d[0])
            nc.gpsimd.dma_start(padt[:, 1], pad_d[1])
            nc.sync.dma_start(hbt[:, 0], hb_d[0])
            nc.gpsimd.dma_start(hbt[:, 1], hb_d[1])

            # ---- stages ----
            def conv_stage(dirc, l, srcset, pair, wt, n_outer=False):
                off0 = 0 if dirc == "f" else WIDTH
                x0 = {}
                if n_outer:  # startup: early groups only need early n-waves
                    groups = [(b, n) for n in range(HC) for b in pair]
                else:
                    groups = [(b, n) for b in pair for n in range(HC)]
                for b, n in groups:
                    if b not in x0:
                        x0[b] = xmid.tile([128, HC, S], FP16, name=f"x0_{b}", tag="xmid", bufs=8)
                    ps = psum.tile([128, S], F32, name=f"cps_{b}_{n}", tag="cpsum", bufs=4)
                    for r in range(NR):
                        k, ci = divmod(r, HC)
                        off = off0 + k
                        nc.tensor.matmul(
                            ps[:],
                            lhsT=wt[:, n, r],
                            rhs=srcset[b][:, ci, off:off + S],
                            start=(r == 0),
                            stop=(r == NR - 1),
                        )
                    nc.vector.tensor_scalar(
                        x0[b][:, n], ps[:], cbias(dirc, l, n), 0.0, ADD, MAX
                    )
                return x0

            def hw_stage(dirc, l, j, srcset, pair, wt, final):
                outs = {}
                last = dirc == "b" and l == L - 1
                for b in pair:
                    if final:
                        if l + 1 < L:
                            at = acts.tile([128, HC, SPAD], FP16,
                                           name=f"a_{dirc}{l}_{b}", tag="acts", bufs=12)
                            write_pads(at, l + 1)
                        else:
                            at = outsp.tile([128, HC, S], FP16,
                                            name=f"o_{dirc}{l}_{b}", tag="outs", bufs=5)
                        outs[b] = at
                    else:
                        outs[b] = xmid.tile([128, HC, S], FP16, name=f"x1_{b}", tag="xmid", bufs=8)
                    for c in range(HC):
                        pnl = psum.tile([128, S], F32, name=f"hnl_{b}_{c}", tag="hpsum", bufs=4)
                        for h in range(HC):
                            nc.tensor.matmul(
                                pnl[:],
                                lhsT=wt[:, h, c * 128:(c + 1) * 128],
                                rhs=srcset[b][:, h],
                                start=(h == 0),
                                stop=(h == HC - 1),
                            )
                        pgt = psum.tile([128, S], F32, name=f"hgt_{b}_{c}", tag="hpsum", bufs=4)
                        for h in range(HC):
                            nc.tensor.matmul(
                                pgt[:],
                                lhsT=wt[:, h, H + c * 128:H + (c + 1) * 128],
                                rhs=srcset[b][:, h],
                                start=(h == 0),
                                stop=(h == HC - 1),
                            )
                        r = work.tile([128, S], FP16, name=f"r_{b}_{c}", tag="r", bufs=4)
                        nc.scalar.activation(r[:], pnl[:], RELU, bias=hbias(dirc, l, j, c))
                        g = work.tile([128, S], FP16, name=f"g_{b}_{c}", tag="g", bufs=4)
                        nc.scalar.activation(g[:], pgt[:], SIGM, bias=hbias(dirc, l, j, HC + c))
                        d = work.tile([128, S], FP16, name=f"d_{b}_{c}", tag="d", bufs=4)
                        nc.vector.tensor_sub(d[:], srcset[b][:, c], r[:])
                        nc.vector.tensor_mul(d[:], g[:], d[:])
                        if final:
                            if l + 1 < L:
                                dst = outs[b][:, c, WIDTH:WIDTH + S]
                            else:
                                dst = outs[b][:, c]
                            nc.vector.tensor_add(dst, d[:], r[:])
                            if last:
                                # fine-grained final drain: store each chunk
                                # as soon as its combine lands
                                QS[(c + b) % len(QS)].dma_start(out_d[l, b, 1][:, c], dst)
                        else:
                            nc.vector.tensor_add(outs[b][:, c], d[:], r[:])
                if final and not last:
                    k = 0 if dirc == "f" else 1
                    for b in pair:
                        src = outs[b][:, :, WIDTH:WIDTH + S] if l + 1 < L else outs[b][:]
                        qnext().dma_start(out_d[l, b, k], src)
                return outs

            # ---- main chain: f fully, then b (xT stays resident for b) ----
            PAIRS = [(0, 1), (2, 3)]
            first = True
            for dirc in ("f", "b"):
                cur = xT
                for l in range(L):
                    cw = ensure_convw(dirc, l)
                    nxt = {}
                    hw = None
                    for pair in PAIRS:
                        x0 = conv_stage(dirc, l, cur, pair, cw, n_outer=first)
                        # hww DMAs emitted after the first conv groups so they
                        # don't compete with the critical startup loads; also
                        # prefetch the next conv weights here.
                        if hw is None:
                            hw = ensure_hww(dirc, l)
                            nl = (dirc, l + 1) if l + 1 < L else ("b", 0)
                            if nl not in loaded_cw:
                                ensure_convw(*nl)
                        x1 = hw_stage(dirc, l, 0, x0, pair, hw[0], final=False)
                        res = hw_stage(dirc, l, 1, x1, pair, hw[1], final=True)
                        nxt.update(res)
                    first = False
                    cur = nxt

    nc.compile()
    return nc


_CACHE = {}


def _get_program():
    if "nc" not in _CACHE:
        _CACHE["nc"] = _build_program()
    return _CACHE["nc"]


def _make_in_maps(inputs):
    f16 = np.float16

    # Conv weights [L, CIN, H] -> [L, 128, n, r, 128]
    def packw(w):
        w = np.asarray(w, dtype=np.float32).astype(f16)
        # CIN index = r*128 + kp ; H index = n*128 + m
        w = w.reshape(L, NR, 128, HC, 128)           # [l, r, kp, n, m]
        return np.ascontiguousarray(w.transpose(0, 2, 3, 1, 4))  # [l, kp, n, r, m]

    # Highway weights [L, NHW, H, 2H] -> [L, NHW, 128, h, 2H]
    def packhw(w):
        w = np.asarray(w, dtype=np.float32).astype(f16)
        w = w.reshape(L, NHW, HC, 128, 2 * H)        # [l, j, h, kp, 2H]
        return np.ascontiguousarray(w.transpose(0, 1, 3, 2, 4))  # [l, j, kp, h, 2H]

    fw = packw(inputs["fwd_W"])
    bw = packw(inputs["bwd_W"])
    fhw = packhw(inputs["fwd_hw_W"])
    bhw = packhw(inputs["bwd_hw_W"])

    # Conv biases [L, H] -> [128, L*HC] stacked f/b
    def packcb(b):
        b = np.asarray(b, dtype=np.float32).reshape(L, HC, 128).transpose(2, 0, 1)
        return b.reshape(128, L * HC)

    cb = np.ascontiguousarray(np.stack([packcb(inputs["fwd_b"]), packcb(inputs["bwd_b"])]))

    # Highway biases [L, NHW, 2H] -> [128, L*NHW*2HC] stacked f/b
    def packhb(b):
        b = np.asarray(b, dtype=np.float32).reshape(L, NHW, 2 * HC, 128).transpose(3, 0, 1, 2)
        return b.reshape(128, L * NHW * 2 * HC)

    hb = np.ascontiguousarray(np.stack([packhb(inputs["fwd_hw_b"]), packhb(inputs["bwd_hw_b"])]))

    # Pads [L, W, H] -> [128, L, HC, W] stacked f/b
    def packpad(p):
        p = np.asarray(p, dtype=np.float32).reshape(L, WIDTH, HC, 128).transpose(3, 0, 2, 1)
        return np.ascontiguousarray(p.astype(f16))   # [128, L, HC, W]

    pad = np.ascontiguousarray(np.stack([packpad(inputs["fwd_pads"]), packpad(inputs["bwd_pads"])]))

    # Layer-0 input: [B, S, H] -> per core [BLOC, 128, HC, SPAD] host-padded
    x = np.asarray(inputs["inputs"], dtype=np.float32).astype(f16)
    fpads0 = np.asarray(inputs["fwd_pads"], dtype=np.float32)[0].astype(f16)  # [W, H]
    bpads0 = np.asarray(inputs["bwd_pads"], dtype=np.float32)[0].astype(f16)

    shared = {
        "fw": fw, "bw": bw, "fhw": fhw, "bhw": bhw,
        "cb": cb, "hb": hb, "pad": pad,
    }
    in_maps = []
    for i in range(NCORES):
        m = dict(shared)
        xi = x[i * BLOC:(i + 1) * BLOC]              # [BLOC, S, H]
        xp = np.empty((BLOC, SPAD, H), dtype=f16)
        xp[:, WIDTH:WIDTH + S] = xi
        xp[:, :WIDTH] = fpads0
        xp[:, WIDTH + S:] = bpads0
        # [BLOC, SPAD, (HC,128)] -> [BLOC, 128, HC, SPAD]
        xp = xp.reshape(BLOC, SPAD, HC, 128).transpose(0, 3, 2, 1)
        m["xT"] = np.ascontiguousarray(xp)
        in_maps.append(m)
    return in_maps


def _run(inputs, trace=False, tmpdir=None):
    nc = _get_program()
    in_maps = _make_in_maps(inputs)
    res = run_bass_kernel_spmd(
        nc, in_maps, core_ids=list(range(NCORES)), trace=trace, tmpdir=tmpdir
    )
    # [L, BLOC, 2, 128, HC, S] per core -> concat on batch -> [L, B, S, 2H] fp32
    out = np.concatenate([np.asarray(r["out"]) for r in res.results], axis=1)
    # [l, b, dir, p, c, t] -> [l, b, t, dir, c, p]
    out = out.transpose(0, 1, 5, 2, 4, 3).reshape(L, B, S, 2 * H).astype(np.float32)
    return out, res


def kernel(**inputs):
    trace = bool(int(os.environ.get("BASS_KERNEL_TRACE", "0")))
    out, _ = _run(inputs, trace=trace)
    return out


# revision 10
# speedup vs baseline: 1.0041x; 1.0028x over previous
"""Trainium2 Bass kernel for the Bengio03 Highway BiLM problem.

Math (see reference): L=3 layers, each with fwd/bwd chains. Per (layer, dir):
  padded = [front_pads(4), seq(512), back_pads(4)]          # [B, 520, H]
  pre[t] = sum_{k=0..4} padded[t + k + off] @ W[k*H:(k+1)*H]  (off=0 fwd, 4 bwd)
  x0 = relu(pre + b)
  2x highway: proj = x @ Ws[j] + bs[j]; nonlin,gate = split(proj)
              x = sigmoid(gate)*x + (1-sigmoid(gate))*relu(nonlin)
  out[l,:,:, 0:512] = f chain, [512:1024] = b chain

Implementation strategy (per core, data-parallel over batch: 4 seqs/core):
  - All matmul operands in fp16 (fp32 PSUM accumulation): same PE rate as
    bf16 (1 cycle/row) but 8x less rounding noise (10-bit vs 7-bit mantissa).
  - Activations kept feature-major in SBUF, one tile per sequence holding all
    4 hidden chunks: [128(h), 4(chunk), 520(t)] (pads inline). Matmuls compute
    outT = W_tile.T @ xT directly (weights stationary lhsT, activations stream
    as rhs with N=512 tokens); the 5-tap conv is 5 shifted rhs slices
    accumulated in PSUM (20 matmuls of [128,128]@[128,512] per output chunk).
  - Layer-0 input arrives host-padded and pre-transposed; output is stored
    feature-major fp16 and re-transposed on the host.
  - DMAs are merged into few large transfers (host pre-packs weights in SBUF
    layout): 4 n-waves per (l,dir) conv tensor, 1 per (l,dir,j) highway
    tensor, 1 per input sequence, 1 per output tile (~75 triggers total,
    ~650ns each on the issuing engine), round-robined over sync/gpsimd so
    scalar/vector queues stay clean.
  - A short warm-up matmul train runs while the first DMAs land so the PE
    clock is at 2.4GHz (not the 1.2GHz cold pstate) when real work starts.
  - Final-layer bwd stores are emitted per chunk so the drain overlaps the
    last compute instead of serializing after it.
  - Elementwise: VectorE does conv relu + the 3 highway-combine ops, ScalarE
    does relu/sigmoid of the highway (+bias, PSUM->SBUF), GpSimd writes pad
    columns for layer>0 activations.
"""

import os
import sys

sys.path.insert(0, "/opt/trn_rl_repo")

import numpy as np

import concourse.bass as bass
import concourse.bacc as bacc
import concourse.tile as tile
from concourse import mybir
from concourse.bass_utils import run_bass_kernel_spmd

# Problem constants (hardcoded per spec).
L = 3
WIDTH = 4
H = 512
B = 32
S = 512
NHW = 2
CIN = (WIDTH + 1) * H  # 2560
NCORES = 8
BLOC = B // NCORES  # 4 sequences per core
HC = H // 128  # 4 hidden chunks of 128
SPAD = S + 2 * WIDTH  # 520
NR = CIN // 128  # 20 contraction chunks for the conv
F32 = mybir.dt.float32
FP16 = mybir.dt.float16
RELU = mybir.ActivationFunctionType.Relu
SIGM = mybir.ActivationFunctionType.Sigmoid
ADD = mybir.AluOpType.add
MAX = mybir.AluOpType.max


def _build_program():
    nc = bacc.Bacc(
        "TRN2",
        target_bir_lowering=False,
        debug=False,
        enable_asserts=False,
        num_devices=1,
    )

    # Layer-0 input, host-padded + feature-major: [b, 128, chunk, 520]
    xT_d = nc.dram_tensor("xT", [BLOC, 128, HC, SPAD], FP16, kind="ExternalInput").ap()
    # Conv weights in SBUF layout: [l, 128, n(out chunk), r(contraction), 128]
    fw_d = nc.dram_tensor("fw", [L, 128, HC, NR, 128], FP16, kind="ExternalInput").ap()
    bw_d = nc.dram_tensor("bw", [L, 128, HC, NR, 128], FP16, kind="ExternalInput").ap()
    # Highway weights: [l, j, 128, h(4), 2H]
    fhw_d = nc.dram_tensor("fhw", [L, NHW, 128, HC, 2 * H], FP16, kind="ExternalInput").ap()
    bhw_d = nc.dram_tensor("bhw", [L, NHW, 128, HC, 2 * H], FP16, kind="ExternalInput").ap()
    # Biases host-packed into single planes: [128, L*HC] / [128, L*NHW*2HC]
    cb_d = nc.dram_tensor("cb", [2, 128, L * HC], F32, kind="ExternalInput").ap()
    hb_d = nc.dram_tensor("hb", [2, 128, L * NHW * 2 * HC], F32, kind="ExternalInput").ap()
    # Pads host-packed: [2, 128, l, chunk, w]
    pad_d = nc.dram_tensor("pad", [2, 128, L, HC, WIDTH], FP16, kind="ExternalInput").ap()
    # Output feature-major fp16: [l, b, dir, 128, chunk, t]; host transposes.
    out_d = nc.dram_tensor("out", [L, BLOC, 2, 128, HC, S], FP16, kind="ExternalOutput").ap()

    with tile.TileContext(nc) as tc:
        with (
            tc.tile_pool(name="consts", bufs=1) as consts,
            tc.tile_pool(name="acts", bufs=12) as acts,
            tc.tile_pool(name="outs", bufs=5) as outsp,
            tc.tile_pool(name="convw", bufs=3) as convw,
            tc.tile_pool(name="hww", bufs=3) as hww,
            tc.tile_pool(name="xmid", bufs=8) as xmid,
            tc.tile_pool(name="work", bufs=4) as work,
            tc.tile_pool(name="psum", bufs=2, space="PSUM") as psum,
        ):
            QS = (nc.sync, nc.gpsimd)
            qctr = [0]

            def qnext():
                q = QS[qctr[0] % len(QS)]
                qctr[0] += 1
                return q

            # ---- PE warm-up: dummy matmuls on a memset tile while the ----
            # ---- critical input/weight DMAs land (cold pstate is 1.2GHz).
            # The train is sized to bridge until the first input+weight
            # DMAs complete (~20us incl. runtime init) so the PE hits the
            # first real matmul already at full clock with no idle gap.
            warm = consts.tile([128, 512], FP16, name="warm", tag="warm", bufs=1)
            nc.gpsimd.memset(warm[:], 0.0)
            wps = psum.tile([128, S], F32, name="wps", tag="cpsum", bufs=4)
            for i in range(20):
                nc.tensor.matmul(
                    wps[:], lhsT=warm[:, 0:128], rhs=warm[:],
                    start=True, stop=True,
                )

            # ---- tiles for constants (DMAs issued in the startup block) ----
            padt = consts.tile([128, 2, L, HC, WIDTH], FP16, name="padt", tag="padt", bufs=1)
            cbt = consts.tile([128, 2, L * HC], F32, name="cbt", tag="cbt", bufs=1)
            hbt = consts.tile([128, 2, L * NHW * 2 * HC], F32, name="hbt", tag="hbt", bufs=1)

            def cbias(dirc, l, n):
                return cbt[:, 0 if dirc == "f" else 1, l * HC + n: l * HC + n + 1]

            def hbias(dirc, l, j, c):
                o = (l * NHW + j) * 2 * HC + c
                return hbt[:, 0 if dirc == "f" else 1, o:o + 1]

            def write_pads(at, l):
                # at: [128, HC, SPAD]; fill cols [0:4] and [516:520] per chunk
                nc.gpsimd.tensor_copy(at[:, :, 0:WIDTH], padt[:, 0, l])
                nc.gpsimd.tensor_copy(at[:, :, WIDTH + S:SPAD], padt[:, 1, l])

            loaded_cw = {}
            loaded_hw = {}

            def load_convw(dirc, l):
                # One tile per n-wave so a conv group (b, n) only depends on
                # the 655KB its own output chunk needs, not the whole 2.6MB.
                src = fw_d if dirc == "f" else bw_d
                waves = []
                for n in range(HC):
                    w = convw.tile([128, NR, 128], FP16,
                                   name=f"cw_{dirc}{l}_{n}", tag="convw", bufs=8)
                    qnext().dma_start(w[:], src[l][:, n])
                    waves.append(w)
                loaded_cw[(dirc, l)] = lambda n, r: waves[n][:, r]
                return loaded_cw[(dirc, l)]

            def ensure_convw(dirc, l):
                if (dirc, l) not in loaded_cw:
                    load_convw(dirc, l)
                return loaded_cw[(dirc, l)]

            def ensure_hww(dirc, l):
                if (dirc, l) not in loaded_hw:
                    src = fhw_d if dirc == "f" else bhw_d
                    jt = []
                    for j in range(NHW):
                        w = hww.tile([128, HC, 2 * H], FP16,
                                     name=f"hw_{dirc}{l}_{j}", tag="hww", bufs=3)
                        qnext().dma_start(w[:], src[l, j])
                        jt.append(w)
                    loaded_hw[(dirc, l)] = jt
                return loaded_hw[(dirc, l)]

            # ---- startup: the first conv group (b0, n0) only needs x0 plus
            # the first 5 contraction chunks of wave n0 (164KB), so wave n0
            # is split into 4 r-subtiles and the critical pieces lead both
            # queues. Remaining waves/inputs stream in while groups run.
            xT = {}
            for b in range(BLOC):
                xT[b] = acts.tile([128, HC, SPAD], FP16, name=f"xT_{b}", tag="acts", bufs=12)
            NSUB = NR // 4  # 5 r-chunks per subtile
            srt = [convw.tile([128, NSUB, 128], FP16, name=f"cw_f0_n0_{i}",
                              tag="cw0", bufs=4) for i in range(4)]
            waves0 = [None]
            for n in range(1, HC):
                waves0.append(convw.tile([128, NR, 128], FP16,
                                         name=f"cw_f0_{n}", tag="convw", bufs=8))
            loaded_cw[("f", 0)] = (
                lambda n, r: srt[r // NSUB][:, r % NSUB] if n == 0 else waves0[n][:, r]
            )
            nc.sync.dma_start(xT[0][:], xT_d[0])
            nc.gpsimd.dma_start(srt[0][:], fw_d[0][:, 0, 0:NSUB])
            nc.sync.dma_start(srt[1][:], fw_d[0][:, 0, NSUB:2 * NSUB])
            nc.gpsimd.dma_start(srt[2][:], fw_d[0][:, 0, 2 * NSUB:3 * NSUB])
            nc.sync.dma_start(srt[3][:], fw_d[0][:, 0, 3 * NSUB:4 * NSUB])
            nc.gpsimd.dma_start(xT[1][:], xT_d[1])
            nc.sync.dma_start(xT[2][:], xT_d[2])
            nc.gpsimd.dma_start(xT[3][:], xT_d[3])
            nc.sync.dma_start(cbt[:, 0], cb_d[0])
            nc.gpsimd.dma_start(cbt[:, 1], cb_d[1])
            nc.sync.dma_start(waves0[1][:], fw_d[0][:, 1])
            nc.gpsimd.dma_start(waves0[2][:], fw_d[0][:, 2])
            nc.sync.dma_start(waves0[3][:], fw_d[0][:, 3])
            nc.gpsimd.dma_start(padt[:, 0], pad_d[0])
            nc.sync.dma_start(padt[:, 1], pad_d[1])
            nc.gpsimd.dma_start(hbt[:, 0], hb_d[0])
            nc.sync.dma_start(hbt[:, 1], hb_d[1])

            # ---- stages ----
            def conv_stage(dirc, l, srcset, pair, wt, n_outer=False):
                off0 = 0 if dirc == "f" else WIDTH
                x0 = {}
                if n_outer:  # startup: early groups only need early n-waves
                    groups = [(b, n) for n in range(HC) for b in pair]
                else:
                    groups = [(b, n) for b in pair for n in range(HC)]
                for b, n in groups:
                    if b not in x0:
                        x0[b] = xmid.tile([128, HC, S], FP16, name=f"x0_{b}", tag="xmid", bufs=8)
                    ps = psum.tile([128, S], F32, name=f"cps_{b}_{n}", tag="cpsum", bufs=4)
                    for r in range(NR):
                        k, ci = divmod(r, HC)
                        off = off0 + k
                        nc.tensor.matmul(
                            ps[:],
                            lhsT=wt(n, r),
                            rhs=srcset[b][:, ci, off:off + S],
                            start=(r == 0),
                            stop=(r == NR - 1),
                        )
                    nc.vector.tensor_scalar(
                        x0[b][:, n], ps[:], cbias(dirc, l, n), 0.0, ADD, MAX
                    )
                return x0

            def hw_stage(dirc, l, j, srcset, pair, wt, final):
                outs = {}
                last = dirc == "b" and l == L - 1
                for b in pair:
                    if final:
                        if l + 1 < L:
                            at = acts.tile([128, HC, SPAD], FP16,
                                           name=f"a_{dirc}{l}_{b}", tag="acts", bufs=12)
                            write_pads(at, l + 1)
                        else:
                            at = outsp.tile([128, HC, S], FP16,
                                            name=f"o_{dirc}{l}_{b}", tag="outs", bufs=5)
                        outs[b] = at
                    else:
                        outs[b] = xmid.tile([128, HC, S], FP16, name=f"x1_{b}", tag="xmid", bufs=8)
                    for c in range(HC):
                        pnl = psum.tile([128, S], F32, name=f"hnl_{b}_{c}", tag="hpsum", bufs=4)
                        for h in range(HC):
                            nc.tensor.matmul(
                                pnl[:],
                                lhsT=wt[:, h, c * 128:(c + 1) * 128],
                                rhs=srcset[b][:, h],
                                start=(h == 0),
                                stop=(h == HC - 1),
                            )
                        pgt = psum.tile([128, S], F32, name=f"hgt_{b}_{c}", tag="hpsum", bufs=4)
                        for h in range(HC):
                            nc.tensor.matmul(
                                pgt[:],
                                lhsT=wt[:, h, H + c * 128:H + (c + 1) * 128],
                                rhs=srcset[b][:, h],
                                start=(h == 0),
                                stop=(h == HC - 1),
                            )
                        r = work.tile([128, S], FP16, name=f"r_{b}_{c}", tag="r", bufs=4)
                        nc.scalar.activation(r[:], pnl[:], RELU, bias=hbias(dirc, l, j, c))
                        g = work.tile([128, S], FP16, name=f"g_{b}_{c}", tag="g", bufs=4)
                        nc.scalar.activation(g[:], pgt[:], SIGM, bias=hbias(dirc, l, j, HC + c))
                        d = work.tile([128, S], FP16, name=f"d_{b}_{c}", tag="d", bufs=4)
                        nc.vector.tensor_sub(d[:], srcset[b][:, c], r[:])
                        nc.vector.tensor_mul(d[:], g[:], d[:])
                        if final:
                            if l + 1 < L:
                                dst = outs[b][:, c, WIDTH:WIDTH + S]
                            else:
                                dst = outs[b][:, c]
                            nc.vector.tensor_add(dst, d[:], r[:])
                            if last:
                                # fine-grained final drain: store each chunk
                                # as soon as its combine lands
                                QS[(c + b) % len(QS)].dma_start(out_d[l, b, 1][:, c], dst)
                        else:
                            nc.vector.tensor_add(outs[b][:, c], d[:], r[:])
                if final and not last:
                    k = 0 if dirc == "f" else 1
                    for b in pair:
                        src = outs[b][:, :, WIDTH:WIDTH + S] if l + 1 < L else outs[b][:]
                        qnext().dma_start(out_d[l, b, k], src)
                return outs

            # ---- main chain: f fully, then b (xT stays resident for b) ----
            PAIRS = [(0, 1), (2, 3)]
            first = True
            for dirc in ("f", "b"):
                cur = xT
                for l in range(L):
                    cw = ensure_convw(dirc, l)
                    nxt = {}
                    hw = None
                    for pair in PAIRS:
                        x0 = conv_stage(dirc, l, cur, pair, cw, n_outer=first)
                        # hww DMAs emitted after the first conv groups so they
                        # don't compete with the critical startup loads; also
                        # prefetch the next conv weights here.
                        if hw is None:
                            hw = ensure_hww(dirc, l)
                            nl = (dirc, l + 1) if l + 1 < L else ("b", 0)
                            if nl not in loaded_cw:
                                ensure_convw(*nl)
                        x1 = hw_stage(dirc, l, 0, x0, pair, hw[0], final=False)
                        res = hw_stage(dirc, l, 1, x1, pair, hw[1], final=True)
                        nxt.update(res)
                    first = False
                    cur = nxt

    nc.compile()
    return nc


_CACHE = {}


def _get_program():
    if "nc" not in _CACHE:
        _CACHE["nc"] = _build_program()
    return _CACHE["nc"]


def _make_in_maps(inputs):
    f16 = np.float16

    # Conv weights [L, CIN, H] -> [L, 128, n, r, 128]
    def packw(w):
        w = np.asarray(w, dtype=np.float32).astype(f16)
        # CIN index = r*128 + kp ; H index = n*128 + m
        w = w.reshape(L, NR, 128, HC, 128)           # [l, r, kp, n, m]
        return np.ascontiguousarray(w.transpose(0, 2, 3, 1, 4))  # [l, kp, n, r, m]

    # Highway weights [L, NHW, H, 2H] -> [L, NHW, 128, h, 2H]
    def packhw(w):
        w = np.asarray(w, dtype=np.float32).astype(f16)
        w = w.reshape(L, NHW, HC, 128, 2 * H)        # [l, j, h, kp, 2H]
        return np.ascontiguousarray(w.transpose(0, 1, 3, 2, 4))  # [l, j, kp, h, 2H]

    fw = packw(inputs["fwd_W"])
    bw = packw(inputs["bwd_W"])
    fhw = packhw(inputs["fwd_hw_W"])
    bhw = packhw(inputs["bwd_hw_W"])

    # Conv biases [L, H] -> [128, L*HC] stacked f/b
    def packcb(b):
        b = np.asarray(b, dtype=np.float32).reshape(L, HC, 128).transpose(2, 0, 1)
        return b.reshape(128, L * HC)

    cb = np.ascontiguousarray(np.stack([packcb(inputs["fwd_b"]), packcb(inputs["bwd_b"])]))

    # Highway biases [L, NHW, 2H] -> [128, L*NHW*2HC] stacked f/b
    def packhb(b):
        b = np.asarray(b, dtype=np.float32).reshape(L, NHW, 2 * HC, 128).transpose(3, 0, 1, 2)
        return b.reshape(128, L * NHW * 2 * HC)

    hb = np.ascontiguousarray(np.stack([packhb(inputs["fwd_hw_b"]), packhb(inputs["bwd_hw_b"])]))

    # Pads [L, W, H] -> [128, L, HC, W] stacked f/b
    def packpad(p):
        p = np.asarray(p, dtype=np.float32).reshape(L, WIDTH, HC, 128).transpose(3, 0, 2, 1)
        return np.ascontiguousarray(p.astype(f16))   # [128, L, HC, W]

    pad = np.ascontiguousarray(np.stack([packpad(inputs["fwd_pads"]), packpad(inputs["bwd_pads"])]))

    # Layer-0 input: [B, S, H] -> per core [BLOC, 128, HC, SPAD] host-padded
    x = np.asarray(inputs["inputs"], dtype=np.float32).astype(f16)
    fpads0 = np.asarray(inputs["fwd_pads"], dtype=np.float32)[0].astype(f16)  # [W, H]
    bpads0 = np.asarray(inputs["bwd_pads"], dtype=np.float32)[0].astype(f16)

    shared = {
        "fw": fw, "bw": bw, "fhw": fhw, "bhw": bhw,
        "cb": cb, "hb": hb, "pad": pad,
    }
    in_maps = []
    for i in range(NCORES):
        m = dict(shared)
        xi = x[i * BLOC:(i + 1) * BLOC]              # [BLOC, S, H]
        xp = np.empty((BLOC, SPAD, H), dtype=f16)
        xp[:, WIDTH:WIDTH + S] = xi
        xp[:, :WIDTH] = fpads0
        xp[:, WIDTH + S:] = bpads0
        # [BLOC, SPAD, (HC,128)] -> [BLOC, 128, HC, SPAD]
        xp = xp.reshape(BLOC, SPAD, HC, 128).transpose(0, 3, 2, 1)
        m["xT"] = np.ascontiguousarray(xp)
        in_maps.append(m)
    return in_maps


def _run(inputs, trace=False, tmpdir=None):
    nc = _get_program()
    in_maps = _make_in_maps(inputs)
    res = run_bass_kernel_spmd(
        nc, in_maps, core_ids=list(range(NCORES)), trace=trace, tmpdir=tmpdir
    )
    # [L, BLOC, 2, 128, HC, S] per core -> concat on batch -> [L, B, S, 2H] fp32
    out = np.concatenate([np.asarray(r["out"]) for r in res.results], axis=1)
    # [l, b, dir, p, c, t] -> [l, b, t, dir, c, p]
    out = out.transpose(0, 1, 5, 2, 4, 3).reshape(L, B, S, 2 * H).astype(np.float32)
    return out, res


def kernel(**inputs):
    trace = bool(int(os.environ.get("BASS_KERNEL_TRACE", "0")))
    out, _ = _run(inputs, trace=trace)
    return out
